# revision 9
# baseline (speedup 1.0000x reference)
"""DLinear forecast model as a single fused matmul on 8 TRN2 NeuronCores.

The model is out[b,p,c] = relu( sum_t seasonal[b,t,c]*Ws[p,t] + bs[p]
                               + sum_t trend[b,t,c]*Wt[p,t]    + bt[p] )
with trend = moving_avg(x) (kernel 5, edge pad) and seasonal = x - trend.
The moving average is a linear map over the time axis: trend = A @ x with
A [336,336].  Folding it into the weights gives a single matmul:
    out = relu(W_eff @ x[b] + bias),  W_eff = Ws + (Wt - Ws) @ A
Sharding: data-parallel over batch (64 = 8 cores x 8).

Schedule: batches processed in pairs with the contraction (k) loop outside
the free-dim loop, so 8 consecutive matmuls share the same stationary
weights.  Explicit InstLdweights are stripped from the module and walrus is
run with --enable-ldw-opt=true, so the codegen emits one weight load per
distinct weight slice instead of one per matmul (72 loads/rep instead of
576), removing the per-matmul LDWEIGHTS overhead on the PE.
"""

import numpy as np
import ml_dtypes

import concourse.bass as bass
import concourse.mybir as mybir
from concourse.tile import TileContext
from concourse import bass_utils
from concourse.bass_utils import run_bass_kernel_spmd

# Problem shapes (hardcoded per contract)
B, T, C = 64, 336, 1782
P_OUT = 720
N_CORES = 8
B_LOC = B // N_CORES  # 8 batches per core
PAIRS = B_LOC // 2    # batches processed in pairs (psum: 2 x 4 banks)

KCH = 112   # contraction chunk (3 chunks of 112 = 336)
NK = 3
MCH = 120   # output-partition chunk (6 chunks of 120 = 720)
NM = 6
NCH = 512   # free-dim chunk (512,512,512,246)
N_SPLITS = [(i * NCH, min(NCH, C - i * NCH)) for i in range((C + NCH - 1) // NCH)]

BF16 = mybir.dt.bfloat16
F32 = mybir.dt.float32


import os

_LDW_MODE = os.environ.get("KERNEL_LDW_MODE", "dedup")  # "dedup" | "off"


def _dedup_ldweights(nc):
    """Delete an InstLdweights whose weights access pattern is identical to
    the previous one on the PE queue (the PE array keeps the stationary
    weights across matmuls, so the reload is redundant).  The tile legalizer
    emits one Ldweights per matmul; with the k loop hoisted outside the
    free-dim chunks, runs of 4 consecutive matmuls share weights, so this
    removes ~3/4 of all weight loads.  A deleted Ldweights' waits move onto
    the following Matmult; loads carrying sem updates are kept."""

    def _w(inst):
        return list(inst.sync_info.on_wait) if inst.sync_info else []

    def _u(inst):
        return list(inst.sync_info.on_update) if inst.sync_info else []

    n_del = 0
    for f in nc.m.functions:
        for bb in f.blocks:
            new = []
            last_key = None
            pending = None
            for inst in bb.instructions:
                if isinstance(inst, mybir.InstLdweights):
                    k = repr(inst.ins[0])
                    if k == last_key and not _u(inst):
                        if _w(inst):
                            assert pending is None
                            pending = inst
                        n_del += 1
                        continue
                    last_key = k
                    new.append(inst)
                    continue
                if isinstance(inst, mybir.InstMatmult):
                    if pending is not None:
                        ws = _w(pending) + _w(inst)
                        us = _u(inst)
                        inst.sync_info = mybir.SyncInfo(on_wait=ws, on_update=us)
                        pending = None
                elif inst.engine == mybir.EngineType.PE and not isinstance(
                    inst, mybir.InstNoOp
                ):
                    # conservative: any other PE instruction invalidates the
                    # loaded-weights assumption
                    last_key = None
                new.append(inst)
            assert pending is None
            bb.instructions = new
    return n_del


def _split_excess_waits(nc, limit=1):
    """walrus in this toolchain rejects >limit sem-waits per instruction; move
    the extras onto injected same-engine NoOps immediately before it (same
    engine queue => program order => semantics preserved)."""
    seq = 0
    for f in nc.m.functions:
        for bb in f.blocks:
            new = []
            for inst in bb.instructions:
                si = inst.sync_info
                if si is not None and si.on_wait and len(si.on_wait) > limit:
                    waits = list(si.on_wait)
                    head, tail = waits[:-limit], waits[-limit:]
                    for w in head:
                        seq += 1
                        nop = mybir.InstNoOp(
                            name=f"{inst.name}-prewait{seq}", engine=inst.engine
                        )
                        nop.sync_info = mybir.SyncInfo(on_wait=[w], on_update=[])
                        new.append(nop)
                    inst.sync_info = mybir.SyncInfo(on_wait=tail, on_update=si.on_update)
                new.append(inst)
            bb.instructions = new


def build_kernel(reps=1):
    nc = bass.Bass()
    x = nc.declare_dram_parameter("x", [B_LOC, T, C], BF16, isOutput=False)
    w = nc.declare_dram_parameter("w", [T, P_OUT], BF16, isOutput=False)
    bias = nc.declare_dram_parameter("bias", [MCH, NM], F32, isOutput=False)
    out = nc.declare_dram_parameter("out", [B_LOC, P_OUT, C], BF16, isOutput=True)

    with TileContext(nc) as tc:
        with (
            tc.tile_pool(name="wpool", bufs=1) as wpool,
            tc.tile_pool(name="bpool", bufs=1) as bpool,
            tc.tile_pool(name="xpool", bufs=2) as xpool,
            tc.tile_pool(name="opool", bufs=6) as opool,
            tc.tile_pool(name="psum", bufs=2, space="PSUM") as pspool,
        ):
            # x loads on the SP HW-DGE ring (FIFO descriptor gen, ~0.6us per
            # dma_start) in consumption order; w/bias on the ACT ring so
            # their descriptor gen runs in parallel during the ramp.
            def load_x(b, k):
                t = xpool.tile([KCH, C], BF16, tag=f"x{k}b{b % 2}")
                nc.sync.dma_start(out=t[:], in_=x[b, k * KCH : (k + 1) * KCH, :])
                return t

            xt_next = [[load_x(0, 0)], [load_x(1, 0)]]
            wt = []
            for k in range(NK):
                t = wpool.tile([KCH, P_OUT], BF16, tag=f"w{k}")
                nc.scalar.dma_start(out=t[:], in_=w[k * KCH : (k + 1) * KCH, :])
                wt.append(t)
            for k in (1, 2):
                xt_next[0].append(load_x(0, k))
                xt_next[1].append(load_x(1, k))
            bt = bpool.tile([MCH, NM], F32)
            nc.scalar.dma_start(out=bt[:], in_=bias[:])

            # PE warmup: dummy matmuls on memset tiles (no DMA deps) fill the
            # initial load wait and lift the HAM clock gate (1.2->2.4 GHz
            # needs ~3.4us of sustained PE activity).
            dz = wpool.tile([KCH, NCH], BF16, tag="warm")
            nc.vector.memset(dz[:], 0.0)
            psw = pspool.tile([MCH, 1024], F32, tag="psH0")
            for _ in range(8):
                nc.tensor.matmul(
                    psw[:, :NCH], dz[:, :MCH], dz[:, :NCH], start=True, stop=True
                )

            # free-dim halves: h0 = [0:1024), h1 = [1024:1782).  Each (m, h)
            # sub-block holds 2-bank psum tiles per batch of the pair (4
            # banks live per sub-block, 8 across the double buffer), and the
            # k loop sits outside (j, chunk) so 4 consecutive matmuls share
            # one stationary-weight load.
            HALVES = [
                [(0, NCH), (NCH, NCH)],
                [(2 * NCH, NCH), (3 * NCH, C - 3 * NCH)],
            ]
            HW_ = [2 * NCH, C - 2 * NCH]  # widths 1024, 758
            n_iters = PAIRS * reps
            for i in range(n_iters):
                p = i % PAIRS
                xt = xt_next  # xt[j][k] = x tile for batch 2p+j, k-chunk k
                xt_next = [[], []]
                for m in range(NM):
                    ot = [
                        opool.tile([MCH, C], BF16, tag="o0", name="otA"),
                        opool.tile([MCH, C], BF16, tag="o1", name="otB"),
                    ]
                    wsl = [wt[k][:, m * MCH : (m + 1) * MCH] for k in range(NK)]
                    for h, chunks in enumerate(HALVES):
                        hoff = 2 * NCH * h
                        ps = [
                            pspool.tile([MCH, HW_[h]], F32, tag=f"psH{h}",
                                        name="psA", padded_shape=[MCH, 1024]),
                            pspool.tile([MCH, HW_[h]], F32, tag=f"psH{h}",
                                        name="psB", padded_shape=[MCH, 1024]),
                        ]
                        for k in range(NK):
                            for j in (0, 1):
                                for noff, nw in chunks:
                                    nc.tensor.matmul(
                                        ps[j][:, noff - hoff : noff - hoff + nw],
                                        wsl[k],
                                        xt[j][k][:, noff : noff + nw],
                                        start=(k == 0),
                                        stop=(k == NK - 1),
                                    )
                        # evict: bias + relu + bf16 cast; j0 on DVE, j1 on ACT
                        nc.vector.tensor_scalar(
                            ot[0][:, hoff : hoff + HW_[h]],
                            ps[0][:],
                            bt[:, m : m + 1],
                            0.0,
                            op0=mybir.AluOpType.add,
                            op1=mybir.AluOpType.max,
                        )
                        nc.scalar.activation(
                            ot[1][:, hoff : hoff + HW_[h]],
                            ps[1][:],
                            mybir.ActivationFunctionType.Relu,
                            bias=bt[:, m : m + 1],
                        )
                    for j in (0, 1):
                        nc.sync.dma_start(
                            out=out[2 * p + j, m * MCH : (m + 1) * MCH, :],
                            in_=ot[j][:],
                        )
                    # prefetch next pair's x tiles, one per m block
                    if i + 1 < n_iters:
                        pn = (i + 1) % PAIRS
                        if m < NK:
                            xt_next[0].append(load_x(2 * pn, m))
                        elif m < 2 * NK:
                            xt_next[1].append(load_x(2 * pn + 1, m - NK))

    if _LDW_MODE == "dedup":
        _dedup_ldweights(nc)
    _split_excess_waits(nc)
    return nc


def host_weights(W_seasonal, b_seasonal, W_trend, b_trend):
    """Fold the moving average into one weight matrix (f64 precision)."""
    K, PAD = 5, 2
    A = np.zeros((T, T), dtype=np.float64)
    idx = np.arange(T)
    for d in range(-PAD, PAD + 1):
        np.add.at(A, (idx, np.clip(idx + d, 0, T - 1)), 1.0 / K)
    Ws = W_seasonal.astype(np.float64)
    Wt = W_trend.astype(np.float64)
    W_eff = Ws + (Wt - Ws) @ A  # [720, 336]
    bias = (b_seasonal.astype(np.float64) + b_trend.astype(np.float64)).astype(
        np.float32
    )
    wT = np.ascontiguousarray(W_eff.T.astype(np.float32)).astype(ml_dtypes.bfloat16)
    bias_tiled = np.ascontiguousarray(bias.reshape(NM, MCH).T)  # [120, 6]
    return wT, bias_tiled


def make_in_maps(x, W_seasonal, b_seasonal, W_trend, b_trend):
    wT, bias_tiled = host_weights(W_seasonal, b_seasonal, W_trend, b_trend)
    xb = np.asarray(x).astype(ml_dtypes.bfloat16)
    return [
        {
            "x": np.ascontiguousarray(xb[i * B_LOC : (i + 1) * B_LOC]),
            "w": wT,
            "bias": bias_tiled,
        }
        for i in range(N_CORES)
    ]


def kernel(x, W_seasonal, b_seasonal, W_trend, b_trend):
    x = np.asarray(x)
    W_seasonal = np.asarray(W_seasonal)
    b_seasonal = np.asarray(b_seasonal)
    W_trend = np.asarray(W_trend)
    b_trend = np.asarray(b_trend)
    in_maps = make_in_maps(x, W_seasonal, b_seasonal, W_trend, b_trend)
    for attempt in range(3):
        try:
            nc = build_kernel()
            res = run_bass_kernel_spmd(nc, in_maps, core_ids=list(range(N_CORES)))
            break
        except Exception:  # transient device wedge (NRT_EXEC_UNIT_...)
            if attempt == 2:
                raise
            import time as _time

            _time.sleep(20)
    parts = [res.results[i]["out"].astype(np.float32) for i in range(N_CORES)]
    return np.concatenate(parts, axis=0)


# revision 11
# speedup vs baseline: 1.1457x; 1.1457x over previous
"""DLinear forecast model as a single fused matmul on 8 TRN2 NeuronCores.

The model is out[b,p,c] = relu( sum_t seasonal[b,t,c]*Ws[p,t] + bs[p]
                               + sum_t trend[b,t,c]*Wt[p,t]    + bt[p] )
with trend = moving_avg(x) (kernel 5, edge pad) and seasonal = x - trend.
The moving average is a linear map over the time axis: trend = A @ x with
A [336,336].  Folding it into the weights gives a single matmul:
    out = relu(W_eff @ x[b] + bias),  W_eff = Ws + (Wt - Ws) @ A
Sharding: data-parallel over batch (64 = 8 cores x 8).

Schedule: batches processed in pairs with the contraction (k) loop outside
the free-dim loop, so 8 consecutive matmuls share the same stationary
weights.  Explicit InstLdweights are stripped from the module and walrus is
run with --enable-ldw-opt=true, so the codegen emits one weight load per
distinct weight slice instead of one per matmul (72 loads/rep instead of
576), removing the per-matmul LDWEIGHTS overhead on the PE.
"""

import numpy as np
import ml_dtypes

import concourse.bass as bass
import concourse.mybir as mybir
from concourse.tile import TileContext
from concourse import bass_utils
from concourse.bass_utils import run_bass_kernel_spmd

# Problem shapes (hardcoded per contract)
B, T, C = 64, 336, 1782
P_OUT = 720
N_CORES = 8
B_LOC = B // N_CORES  # 8 batches per core
PAIRS = B_LOC // 2    # batches processed in pairs (psum: 2 x 4 banks)

KCH = 112   # contraction chunk (3 chunks of 112 = 336)
NK = 3
MCH = 120   # output-partition chunk (6 chunks of 120 = 720)
NM = 6
NCH = 512   # free-dim chunk (512,512,512,246)
N_SPLITS = [(i * NCH, min(NCH, C - i * NCH)) for i in range((C + NCH - 1) // NCH)]

BF16 = mybir.dt.bfloat16
F32 = mybir.dt.float32


import os

_LDW_MODE = os.environ.get("KERNEL_LDW_MODE", "dedup")  # "dedup" | "off"


def _dedup_ldweights(nc):
    """Delete an InstLdweights whose weights access pattern is identical to
    the previous one on the PE queue (the PE array keeps the stationary
    weights across matmuls, so the reload is redundant).  The tile legalizer
    emits one Ldweights per matmul; with the k loop hoisted outside the
    free-dim chunks, runs of 4 consecutive matmuls share weights, so this
    removes ~3/4 of all weight loads.  A deleted Ldweights' waits move onto
    the following Matmult; loads carrying sem updates are kept."""

    def _w(inst):
        return list(inst.sync_info.on_wait) if inst.sync_info else []

    def _u(inst):
        return list(inst.sync_info.on_update) if inst.sync_info else []

    n_del = 0
    for f in nc.m.functions:
        for bb in f.blocks:
            new = []
            last_key = None
            pending = None
            for inst in bb.instructions:
                if isinstance(inst, mybir.InstLdweights):
                    k = repr(inst.ins[0])
                    if k == last_key and not _u(inst):
                        if _w(inst):
                            assert pending is None
                            pending = inst
                        n_del += 1
                        continue
                    last_key = k
                    new.append(inst)
                    continue
                if isinstance(inst, mybir.InstMatmult):
                    if pending is not None:
                        ws = _w(pending) + _w(inst)
                        us = _u(inst)
                        inst.sync_info = mybir.SyncInfo(on_wait=ws, on_update=us)
                        pending = None
                elif inst.engine == mybir.EngineType.PE and not isinstance(
                    inst, mybir.InstNoOp
                ):
                    # conservative: any other PE instruction invalidates the
                    # loaded-weights assumption
                    last_key = None
                new.append(inst)
            assert pending is None
            bb.instructions = new
    return n_del


def _split_excess_waits(nc, limit=1):
    """walrus in this toolchain rejects >limit sem-waits per instruction; move
    the extras onto injected same-engine NoOps immediately before it (same
    engine queue => program order => semantics preserved)."""
    seq = 0
    for f in nc.m.functions:
        for bb in f.blocks:
            new = []
            for inst in bb.instructions:
                si = inst.sync_info
                if si is not None and si.on_wait and len(si.on_wait) > limit:
                    waits = list(si.on_wait)
                    head, tail = waits[:-limit], waits[-limit:]
                    for w in head:
                        seq += 1
                        nop = mybir.InstNoOp(
                            name=f"{inst.name}-prewait{seq}", engine=inst.engine
                        )
                        nop.sync_info = mybir.SyncInfo(on_wait=[w], on_update=[])
                        new.append(nop)
                    inst.sync_info = mybir.SyncInfo(on_wait=tail, on_update=si.on_update)
                new.append(inst)
            bb.instructions = new


def build_kernel(reps=1):
    nc = bass.Bass()
    x = nc.declare_dram_parameter("x", [B_LOC, T, C], BF16, isOutput=False)
    w = nc.declare_dram_parameter("w", [T, P_OUT], BF16, isOutput=False)
    bias = nc.declare_dram_parameter("bias", [MCH, NM], F32, isOutput=False)
    out = nc.declare_dram_parameter("out", [B_LOC, P_OUT, C], BF16, isOutput=True)

    with TileContext(nc) as tc:
        with (
            tc.tile_pool(name="wpool", bufs=1) as wpool,
            tc.tile_pool(name="bpool", bufs=1) as bpool,
            tc.tile_pool(name="xpool", bufs=3) as xpool,
            tc.tile_pool(name="opool", bufs=6) as opool,
            tc.tile_pool(name="psum", bufs=4, space="PSUM") as pspool,
        ):
            # x loads on the SP HW-DGE ring (FIFO descriptor gen, ~0.6us per
            # dma_start) in consumption order; w/bias on the ACT ring so
            # their descriptor gen runs in parallel during the ramp.
            def load_x(b, k):
                t = xpool.tile([KCH, C], BF16, tag=f"x{k}")
                nc.sync.dma_start(out=t[:], in_=x[b, k * KCH : (k + 1) * KCH, :])
                return t

            xt_next = [load_x(0, 0)]
            wt = []
            for k in range(NK):
                t = wpool.tile([KCH, P_OUT], BF16, tag=f"w{k}")
                nc.scalar.dma_start(out=t[:], in_=w[k * KCH : (k + 1) * KCH, :])
                wt.append(t)
            xt_next += [load_x(0, 1), load_x(0, 2)]
            bt = bpool.tile([MCH, NM], F32)
            nc.scalar.dma_start(out=bt[:], in_=bias[:])

            # PE warmup: dummy matmuls on memset tiles (no DMA deps) fill the
            # initial load wait and lift the HAM clock gate (1.2->2.4 GHz
            # needs ~3.4us of sustained PE activity).
            dz = wpool.tile([KCH, NCH], BF16, tag="warm")
            nc.vector.memset(dz[:], 0.0)
            psw = pspool.tile([MCH, 2 * NCH], F32, tag="ps")
            for _ in range(8):
                nc.tensor.matmul(
                    psw[:, :NCH], dz[:, :MCH], dz[:, :NCH], start=True, stop=True
                )

            # psA covers free-dim [0:1024), psB [1024:1782).  The k loop sits
            # outside the 4 free-dim chunks, so 4 consecutive matmuls share
            # the same stationary weights and _dedup_ldweights removes 3 of
            # the 4 weight loads.  bufs=4 on the psum pool keeps the bank
            # reuse distance at 2 blocks so the psum-free waits on the k=0
            # matmuls are long satisfied by the time the PE reaches them.
            for i in range(B_LOC * reps):
                b = i % B_LOC
                xt = xt_next
                xt_next = []
                for m in range(NM):
                    ot = opool.tile([MCH, C], BF16, tag="o")
                    psA = pspool.tile([MCH, 2 * NCH], F32, tag="ps")
                    psB = pspool.tile([MCH, 2 * NCH], F32, tag="ps")
                    for k in range(NK):
                        w_m = wt[k][:, m * MCH : (m + 1) * MCH]
                        st, sp = k == 0, k == NK - 1
                        nc.tensor.matmul(
                            psA[:, 0:NCH], w_m, xt[k][:, 0:NCH], start=st, stop=sp
                        )
                        nc.tensor.matmul(
                            psA[:, NCH : 2 * NCH], w_m, xt[k][:, NCH : 2 * NCH],
                            start=st, stop=sp,
                        )
                        nc.tensor.matmul(
                            psB[:, 0:NCH], w_m, xt[k][:, 2 * NCH : 3 * NCH],
                            start=st, stop=sp,
                        )
                        nc.tensor.matmul(
                            psB[:, NCH : C - 2 * NCH], w_m, xt[k][:, 3 * NCH : C],
                            start=st, stop=sp,
                        )
                    # bias + relu + bf16 cast; DVE takes the 1024-col half,
                    # ACT the 758-col half
                    nc.vector.tensor_scalar(
                        ot[:, : 2 * NCH],
                        psA[:],
                        bt[:, m : m + 1],
                        0.0,
                        op0=mybir.AluOpType.add,
                        op1=mybir.AluOpType.max,
                    )
                    nc.scalar.activation(
                        ot[:, 2 * NCH : C],
                        psB[:, : C - 2 * NCH],
                        mybir.ActivationFunctionType.Relu,
                        bias=bt[:, m : m + 1],
                    )
                    nc.sync.dma_start(
                        out=out[b, m * MCH : (m + 1) * MCH, :], in_=ot[:]
                    )
                    if m < NK and i + 1 < B_LOC * reps:
                        xt_next.append(load_x((i + 1) % B_LOC, m))

    if _LDW_MODE == "dedup":
        _dedup_ldweights(nc)
    _split_excess_waits(nc)
    return nc


def host_weights(W_seasonal, b_seasonal, W_trend, b_trend):
    """Fold the moving average into one weight matrix (f64 precision)."""
    K, PAD = 5, 2
    A = np.zeros((T, T), dtype=np.float64)
    idx = np.arange(T)
    for d in range(-PAD, PAD + 1):
        np.add.at(A, (idx, np.clip(idx + d, 0, T - 1)), 1.0 / K)
    Ws = W_seasonal.astype(np.float64)
    Wt = W_trend.astype(np.float64)
    W_eff = Ws + (Wt - Ws) @ A  # [720, 336]
    bias = (b_seasonal.astype(np.float64) + b_trend.astype(np.float64)).astype(
        np.float32
    )
    wT = np.ascontiguousarray(W_eff.T.astype(np.float32)).astype(ml_dtypes.bfloat16)
    bias_tiled = np.ascontiguousarray(bias.reshape(NM, MCH).T)  # [120, 6]
    return wT, bias_tiled


def make_in_maps(x, W_seasonal, b_seasonal, W_trend, b_trend):
    wT, bias_tiled = host_weights(W_seasonal, b_seasonal, W_trend, b_trend)
    xb = np.asarray(x).astype(ml_dtypes.bfloat16)
    return [
        {
            "x": np.ascontiguousarray(xb[i * B_LOC : (i + 1) * B_LOC]),
            "w": wT,
            "bias": bias_tiled,
        }
        for i in range(N_CORES)
    ]


def kernel(x, W_seasonal, b_seasonal, W_trend, b_trend):
    x = np.asarray(x)
    W_seasonal = np.asarray(W_seasonal)
    b_seasonal = np.asarray(b_seasonal)
    W_trend = np.asarray(W_trend)
    b_trend = np.asarray(b_trend)
    in_maps = make_in_maps(x, W_seasonal, b_seasonal, W_trend, b_trend)
    for attempt in range(3):
        try:
            nc = build_kernel()
            res = run_bass_kernel_spmd(nc, in_maps, core_ids=list(range(N_CORES)))
            break
        except Exception:  # transient device wedge (NRT_EXEC_UNIT_...)
            if attempt == 2:
                raise
            import time as _time

            _time.sleep(20)
    parts = [res.results[i]["out"].astype(np.float32) for i in range(N_CORES)]
    return np.concatenate(parts, axis=0)
